# revision 51
# baseline (speedup 1.0000x reference)
"""Local (windowed) attention with shared KV head — TRN2 Bass kernel.

Problem: b=1, L=4096, d_model=1024, n_head=16, d_head=64, w=512.
  qp = (q@Wq)/8; k,v = kv@Wkv; per 512-chunk attention over {prev,self,next}
  chunks with zero-padded edges (softmax includes exp(0)=1 terms for pads);
  out = ctx @ Wo.

Sharding: sequence-parallel over the 8 chunks, one chunk per NeuronCore.
Each core recomputes the K/V projection for its 3-chunk halo (no
collectives). Edge cores receive zero-filled halo slices, which reproduces
the reference's zero-padding exactly (scores 0 -> exp 1 in the softmax).

All matmuls bf16 (1 cycle/row at 2.4 GHz; fp32 PSUM accumulation). The
steady state is ScalarE-paced: one exp ACTIVATE [128,1024] per y-tile
(~1.15us) against ~0.65us of PE work, so the kernel is structured to keep
the ACTIVATE stream gapless:
  - dummy warmup matmuls open the HAM clock gate during the DMA fill
  - input DMAs spread over 3 queues (sync / scalar / gpsimd)
  - kv-proj and q-proj interleave; q-proj tiles m>=2 are computed inside
    the attention loop (2 pairs ahead) so attention starts ~15us in
  - scores for a head pair interleave into one psum tile; the two matmuls
    use disjoint PE row groups and run concurrently
  - softmax denominators batch through one vector.reciprocal per batch
    (heads 0-11 while pairs 6-7 run; heads 12-15 overlap the out-proj)
  - out-proj runs i-outer in two 4-bank halves right behind the ctxn muls
"""

import numpy as np

B, L, DM, NH, DH, W = 1, 4096, 1024, 16, 64, 512
NCORES = 8
CH = L // NCORES        # 512 tokens per core
YW = 3 * W              # 1536 halo positions
P = 128
NF = DM // P            # 8 feature tiles
NY = YW // P            # 12 y tiles
NPAIR = NH // 2         # 8 head pairs

_CACHE = {}


def _zrow(hh):
    # heads 12,13 -> partitions 32,33 and heads 14,15 -> 64,65 so each
    # late reciprocal batch starts at a 32-aligned partition base
    if hh < 12:
        return hh
    if hh < 14:
        return 32 + (hh - 12)
    return 64 + (hh - 14)


def _build():
    import concourse.mybir as mybir
    import concourse.tile as tile
    from concourse import bacc
    from concourse.masks import make_identity
    from contextlib import ExitStack

    F32 = mybir.dt.float32
    BF16 = mybir.dt.bfloat16
    EXP = mybir.ActivationFunctionType.Exp

    nc = bacc.Bacc("TRN2", target_bir_lowering=False, debug=False)
    QT = nc.dram_tensor("QT", [DM, CH], BF16, kind="ExternalInput")
    ESEL = nc.dram_tensor("ESEL", [66, NH * 64], BF16, kind="ExternalInput")
    KVT = nc.dram_tensor("KVT", [DM, YW], BF16, kind="ExternalInput")
    WQ = nc.dram_tensor("WQ", [DM, DM], BF16, kind="ExternalInput")    # pre-scaled by 1/8
    WVK = nc.dram_tensor("WVK", [DM, P], BF16, kind="ExternalInput")   # [Wv | Wk]
    WO = nc.dram_tensor("WO", [DM, DM], BF16, kind="ExternalInput")
    OUT = nc.dram_tensor("OUT", [CH, DM], F32, kind="ExternalOutput")

    with tile.TileContext(nc) as tc, ExitStack() as ctx:
        perm = ctx.enter_context(tc.tile_pool(name="perm", bufs=1))

        # warmup tile memset first so the dummy matmuls start immediately
        wtile = perm.tile([P, W], BF16, tag="wtile")
        nc.vector.memset(wtile[:], 1.0)
        identb = perm.tile([64, 64], F32, tag="identb")
        make_identity(nc, identb[:])
        esel = perm.tile([66, NH * 64], BF16, tag="esel")

        # --- persistent SBUF tiles
        wvk = [perm.tile([P, P], BF16, tag=f"wvk{f}", name=f"wvk{f}") for f in range(NF)]
        wq = [perm.tile([P, DM], BF16, tag=f"wq{f}", name=f"wq{f}") for f in range(NF)]
        wo = [perm.tile([P, DM], BF16, tag=f"wo{f}", name=f"wo{f}") for f in range(NF)]
        k3T2 = perm.tile([P, YW], BF16, tag="k3T2")
        vTs = perm.tile([64, YW], F32, tag="vTs")
        v65 = [perm.tile([P, 65], BF16, tag=f"v65_{t}", name=f"v65_{t}") for t in range(NY)]
        qpT = [perm.tile([P, CH], BF16, tag=f"qpT{m}", name=f"qpT{m}") for m in range(NF)]
        ctxn = [perm.tile([P, CH], BF16, tag=f"ctxn{i}", name=f"ctxn{i}") for i in range(NPAIR)]
        cxs = [perm.tile([64, W], BF16, tag=f"cxs{h}", name=f"cxs{h}") for h in range(NH)]
        zr16 = perm.tile([66, W], F32, tag="zr16")
        zi16 = perm.tile([66, W], F32, tag="zi16")
        zi16b = perm.tile([66, W], BF16, tag="zi16b")

        # HAM warmup: dense dummy matmuls during the DMA fill open the PE
        # clock gate (needs ~3.4us of sustained activity) and bridge the
        # idle window until the first input tiles land
        with tc.tile_pool(name="wmps", bufs=1, space="PSUM") as wmp:
            wps = wmp.tile([P, W], F32, tag="wm")
            for _ in range(12):
                nc.tensor.matmul(wps[:], wtile[:, 0:P], wtile[:],
                                 start=True, stop=True)

        with tc.tile_pool(name="qt", bufs=1) as qtp, \
             tc.tile_pool(name="qpps", bufs=1, space="PSUM") as qpp, \
             tc.tile_pool(name="zn", bufs=6) as znp:
            qt = [qtp.tile([P, CH], BF16, tag=f"qt{f}", name=f"qt{f}") for f in range(NF)]

            # --- input DMA schedule. The scalar hwdge queue must stay nearly
            # empty: queued DMAs block the exp ACTIVATE stream (in-order
            # queue), so it gets only the 12 early kvt loads.
            #   sync   : wvk, kvt f0-3 (w-chunk ordered), wq-h0
            #   scalar : kvt f4-7 (w-chunk ordered)
            #   gpsimd : qt, esel, wq-h1, wo
            for f in range(NF):
                nc.gpsimd.dma_start(qt[f][:], QT.ap()[P * f:P * (f + 1), :])
            nc.gpsimd.dma_start(esel[:], ESEL.ap()[:, :])
            for f in range(NF):
                nc.gpsimd.dma_start(wq[f][:, W:2 * W],
                                    WQ.ap()[P * f:P * (f + 1), W:2 * W])
            for f in range(NF):
                nc.gpsimd.dma_start(wo[f][:], WO.ap()[P * f:P * (f + 1), :])

            def qproj(m):
                ps = qpp.tile([P, CH], F32, tag="qp")
                for f in range(NF):
                    nc.tensor.matmul(ps[:], wq[f][:, P * m:P * (m + 1)], qt[f][:],
                                     start=(f == 0), stop=(f == NF - 1))
                with nc.allow_low_precision(reason="bf16 attention pipeline"):
                    nc.vector.tensor_copy(qpT[m][:], ps[:])

            with tc.tile_pool(name="kvt", bufs=1) as kvtp, \
                 tc.tile_pool(name="ph0ps", bufs=3, space="PSUM") as ph0, \
                 tc.tile_pool(name="tpps", bufs=2, space="PSUM") as tpp:
                kvt = [kvtp.tile([P, YW], BF16, tag=f"kvt{f}", name=f"kvt{f}")
                       for f in range(NF)]
                for f in range(NF):
                    nc.sync.dma_start(wvk[f][:], WVK.ap()[P * f:P * (f + 1), :])
                # kv halo loads split across sync/scalar, ordered by w-chunk
                for n in range(3):
                    ns_ = slice(W * n, W * (n + 1))
                    for f in range(NF // 2):
                        nc.sync.dma_start(kvt[f][:, ns_],
                                          KVT.ap()[P * f:P * (f + 1), ns_])
                    for f in range(NF // 2, NF):
                        nc.scalar.dma_start(kvt[f][:, ns_],
                                            KVT.ap()[P * f:P * (f + 1), ns_])
                # first wq half behind the kv path on sync
                for f in range(NF):
                    nc.sync.dma_start(wq[f][:, 0:W],
                                      WQ.ap()[P * f:P * (f + 1), 0:W])

                def kvproj(n):
                    ps = ph0.tile([P, W], F32, tag="kvp")
                    for f in range(NF):
                        nc.tensor.matmul(ps[:], wvk[f][:],
                                         kvt[f][:, W * n:W * (n + 1)],
                                         start=(f == 0), stop=(f == NF - 1))
                    ns = slice(W * n, W * (n + 1))
                    with nc.allow_low_precision(reason="bf16 attention pipeline"):
                        nc.vector.tensor_copy(vTs[:, ns], ps[0:64, :])
                        nc.vector.tensor_copy(k3T2[64:128, ns], ps[64:128, :])

                # interleave kv-proj w-chunks with the first q-proj tiles
                kvproj(0)
                kvproj(1)
                kvproj(2)
                # duplicate kT into the low partition half (partition remap)
                nc.sync.dma_start(k3T2[0:64, :], k3T2[64:128, :])
                # v65 tiles: PE transpose of vT + ones column
                for t in range(NY):
                    tp = tpp.tile([P, 64], F32, tag="tp")
                    nc.tensor.transpose(tp[:], vTs[:, P * t:P * (t + 1)],
                                        identb[:])
                    with nc.allow_low_precision(reason="bf16 attention pipeline"):
                        nc.vector.tensor_copy(v65[t][:, 0:64], tp[:])
                    nc.vector.memset(v65[t][:, 64:65], 1.0)
                qproj(0)
                qproj(1)

            def z_recip(heads):
                lo, hi = _zrow(heads[0]), _zrow(heads[-1]) + 1
                with nc.allow_low_precision(reason="softmax denom"):
                    nc.vector.reciprocal(zi16[lo:hi, :], zr16[lo:hi, :])
                    nc.vector.tensor_copy(zi16b[lo:hi, :], zi16[lo:hi, :])

            def z_apply(heads):
                lo, hi = _zrow(heads[0]), _zrow(heads[-1]) + 1
                for hh in heads:
                    i, h = hh // 2, hh % 2
                    zb = qpp.tile([P, W], F32, tag="qp")
                    nc.tensor.matmul(zb[0:64, :],
                                     esel[lo:hi, 64 * hh:64 * (hh + 1)],
                                     zi16b[lo:hi, :], start=True, stop=True)
                    if h == 0:
                        with nc.allow_low_precision(reason="bf16 ctx"):
                            nc.vector.tensor_mul(ctxn[i][0:64, :], cxs[hh][:],
                                                 zb[0:64, :])
                    else:
                        cbt = znp.tile([64, W], BF16, tag="cbt")
                        with nc.allow_low_precision(reason="bf16 ctx"):
                            nc.vector.tensor_mul(cbt[:], cxs[hh][:], zb[0:64, :])
                        nc.sync.dma_start(ctxn[i][64:128, :], cbt[:])

            # --- attention per head pair; per y-tile one [128,1024] psum
            # holds both heads' scores (disjoint PE row groups -> the two
            # matmuls run concurrently), one exp ACTIVATE consumes it
            attn = ExitStack()
            scp = attn.enter_context(tc.tile_pool(name="scps", bufs=2, space="PSUM"))
            cxp = attn.enter_context(tc.tile_pool(name="cxps", bufs=3, space="PSUM"))
            ptp = attn.enter_context(tc.tile_pool(name="pt", bufs=4))
            for i in range(NPAIR):
                cxA = cxp.tile([P, W], F32, tag="cx")
                cxB = cxp.tile([P, W], F32, tag="cx")
                for y in range(NY):
                    ys = slice(P * y, P * (y + 1))
                    sc = scp.tile([P, 2 * W], F32, tag="sc")
                    nc.tensor.matmul(sc[:, 0:W], k3T2[0:64, ys],
                                     qpT[i][0:64, :], start=True, stop=True,
                                     tile_position=(0, 0))
                    nc.tensor.matmul(sc[:, W:2 * W], k3T2[64:128, ys],
                                     qpT[i][64:128, :], start=True, stop=True,
                                     tile_position=(64, 0))
                    pab = ptp.tile([P, 2 * W], BF16, tag="pt")
                    with nc.allow_low_precision(reason="bf16 probs"):
                        nc.scalar.activation(pab[:], sc[:], EXP)
                    st = (y == 0)
                    sp = (y == NY - 1)
                    nc.tensor.matmul(cxA[0:65, :], v65[y][:], pab[:, 0:W],
                                     start=st, stop=sp)
                    nc.tensor.matmul(cxB[0:65, :], v65[y][:], pab[:, W:2 * W],
                                     start=st, stop=sp)
                # stage Z row + unnormalized ctx out of PSUM (frees cx banks)
                for h, cx in ((0, cxA), (1, cxB)):
                    hh = 2 * i + h
                    zt = znp.tile([65, W], F32, tag="zt")
                    nc.vector.tensor_copy(zt[64:65, :], cx[64:65, :])
                    nc.sync.dma_start(zr16[_zrow(hh):_zrow(hh) + 1, :],
                                      zt[64:65, :])
                    with nc.allow_low_precision(reason="bf16 ctx"):
                        nc.vector.tensor_copy(cxs[hh][:], cx[0:64, :])
                if i + 2 < NF:
                    qproj(i + 2)
                if i == 5:
                    z_recip(list(range(12)))     # overlaps pair 6
                if i == 6:
                    z_apply(list(range(12)))     # overlaps pair 7
                    z_recip([12, 13])
                    z_apply([12, 13])
            attn.close()
            # heads 14,15: reciprocal overlaps the first out-proj matmuls
            z_recip([14, 15])

            # --- output projection in two 4-bank halves, i-outer; pairs 6-7
            # normalize while the i<6 matmuls stream
            with tc.tile_pool(name="opps", bufs=4, space="PSUM") as opp, \
                 tc.tile_pool(name="osb", bufs=4) as osb:
                allblk = [(x, o) for x in range(4) for o in range(2)]

                def oproj(pso, blocks, irange):
                    for i in irange:
                        for ps, (x, o) in zip(pso, blocks):
                            xs = slice(P * x, P * (x + 1))
                            os_ = slice(W * o, W * (o + 1))
                            nc.tensor.matmul(ps[:], ctxn[i][:, xs],
                                             wo[i][:, os_],
                                             start=(i == 0),
                                             stop=(i == NPAIR - 1))

                def drain(pso, blocks):
                    # alternate copy engines and DMA queues so the final
                    # 2MB store isn't serialized on one ring
                    for b, (ps, (x, o)) in enumerate(zip(pso, blocks)):
                        ot = osb.tile([P, W], F32, tag="os", name=f"ot{x}_{o}")
                        if b % 2 == 0:
                            nc.scalar.copy(ot[:], ps[:])
                        else:
                            nc.vector.tensor_copy(ot[:], ps[:])
                        q = (nc.sync, nc.scalar)[b % 2]
                        q.dma_start(OUT.ap()[P * x:P * (x + 1),
                                             W * o:W * (o + 1)], ot[:])

                blocks0 = allblk[0:4]
                pso0 = [opp.tile([P, W], F32, tag="op", name=f"op0_{b}")
                        for b in range(4)]
                oproj(pso0, blocks0, range(7))
                z_apply([14, 15])
                oproj(pso0, blocks0, range(7, NPAIR))
                drain(pso0, blocks0)
                blocks1 = allblk[4:8]
                pso1 = [opp.tile([P, W], F32, tag="op", name=f"op1_{b}")
                        for b in range(4)]
                oproj(pso1, blocks1, range(NPAIR))
                drain(pso1, blocks1)

    nc.compile()
    return nc


def _get_nc():
    if "nc" not in _CACHE:
        _CACHE["nc"] = _build()
    return _CACHE["nc"]


def _esel():
    import ml_dtypes
    e = np.zeros((66, NH * 64), ml_dtypes.bfloat16)
    for h in range(NH):
        e[_zrow(h), 64 * h:64 * (h + 1)] = 1.0
    return e


def kernel(q, kv, Wq, Wkv, Wo, w=None, _trace=False):
    from concourse import bass_utils
    import ml_dtypes

    BF = ml_dtypes.bfloat16

    q = np.asarray(q, np.float32).reshape(L, DM)
    kv = np.asarray(kv, np.float32).reshape(L, DM)
    Wq = np.asarray(Wq, np.float32)
    Wkv = np.asarray(Wkv, np.float32)
    Wo = np.asarray(Wo, np.float32)

    qT = np.ascontiguousarray(q.T).astype(BF)           # [DM, L]
    kvT = np.ascontiguousarray(kv.T).astype(BF)         # [DM, L]
    WQs = np.ascontiguousarray(Wq / np.sqrt(DH)).astype(BF)   # fold 1/sqrt(d_head)
    WVK = np.ascontiguousarray(
        np.concatenate([Wkv[:, DH:], Wkv[:, :DH]], axis=1)).astype(BF)  # [Wv | Wk]
    WOb = np.ascontiguousarray(Wo).astype(BF)

    in_maps = []
    for c in range(NCORES):
        kvt_c = np.zeros((DM, YW), BF)
        lo = (c - 1) * CH
        hi = (c + 2) * CH
        src_lo, src_hi = max(lo, 0), min(hi, L)
        dst_lo = src_lo - lo
        kvt_c[:, dst_lo:dst_lo + (src_hi - src_lo)] = kvT[:, src_lo:src_hi]
        in_maps.append({
            "QT": np.ascontiguousarray(qT[:, c * CH:(c + 1) * CH]),
            "KVT": kvt_c,
            "WQ": WQs,
            "WVK": WVK,
            "WO": WOb,
            "ESEL": _esel(),
        })

    nc = _get_nc()
    res = bass_utils.run_bass_kernel_spmd(
        nc, in_maps, core_ids=list(range(NCORES)), trace=_trace)
    if _trace:
        _CACHE["last_result"] = res

    out = np.concatenate([r["OUT"] for r in res.results], axis=0)
    return out.reshape(B, L, DM).astype(np.float32)


# revision 52
# speedup vs baseline: 1.1550x; 1.1550x over previous
"""Local (windowed) attention with shared KV head — TRN2 Bass kernel.

Problem: b=1, L=4096, d_model=1024, n_head=16, d_head=64, w=512.
  qp = (q@Wq)/8; k,v = kv@Wkv; per 512-chunk attention over {prev,self,next}
  chunks with zero-padded edges (softmax includes exp(0)=1 terms for pads);
  out = ctx @ Wo.

Sharding: sequence-parallel over the 8 chunks, one chunk per NeuronCore.
Each core recomputes the K/V projection for its 3-chunk halo (no
collectives). Edge cores receive zero-filled halo slices, which reproduces
the reference's zero-padding exactly (scores 0 -> exp 1 in the softmax).

All matmuls bf16 (1 cycle/row at 2.4 GHz; fp32 PSUM accumulation). The
steady state is ScalarE-paced: one exp ACTIVATE [128,1024] per y-tile
(~1.15us) against ~0.65us of PE work, so the kernel is structured to keep
the ACTIVATE stream gapless:
  - all inputs host-packed partition-major so each loads in ONE DMA
    (DMA queues pace at ~1us/instruction; instruction count, not bytes,
    dominated the old startup)
  - the scalar hwdge queue stays empty until the tail (queued DMAs block
    the exp ACTIVATE stream)
  - dummy warmup matmuls open the HAM clock gate during the DMA fill
  - q-proj tiles m>=2 are computed inside the attention loop, 2 pairs ahead
  - scores for a head pair interleave into one psum tile; the two matmuls
    use disjoint PE row groups and run concurrently
  - softmax denominators batch through one vector.reciprocal per batch
    (heads 0-11 / 12-13 during attention; 14-15 overlap the out-proj)
  - out-proj runs i-outer in two 4-bank halves right behind the ctxn muls
"""

import numpy as np

B, L, DM, NH, DH, W = 1, 4096, 1024, 16, 64, 512
NCORES = 8
CH = L // NCORES        # 512 tokens per core
YW = 3 * W              # 1536 halo positions
P = 128
NF = DM // P            # 8 feature tiles
NY = YW // P            # 12 y tiles
NPAIR = NH // 2         # 8 head pairs

_CACHE = {}


def _zrow(hh):
    # heads 12,13 -> partitions 32,33 and heads 14,15 -> 64,65 so each
    # late reciprocal batch starts at a 32-aligned partition base
    if hh < 12:
        return hh
    if hh < 14:
        return 32 + (hh - 12)
    return 64 + (hh - 14)


def _build():
    import concourse.mybir as mybir
    import concourse.tile as tile
    from concourse import bacc
    from concourse.masks import make_identity
    from contextlib import ExitStack

    F32 = mybir.dt.float32
    BF16 = mybir.dt.bfloat16
    EXP = mybir.ActivationFunctionType.Exp

    nc = bacc.Bacc("TRN2", target_bir_lowering=False, debug=False)
    # host-packed partition-major inputs: X[p, blk*w + c] = orig[128*blk + p, c]
    QTP = nc.dram_tensor("QTP", [P, NF * CH], BF16, kind="ExternalInput")
    KVTP = nc.dram_tensor("KVTP", [P, 3 * NF * W], BF16, kind="ExternalInput")
    WQP = nc.dram_tensor("WQP", [P, NF * DM], BF16, kind="ExternalInput")
    WVKP = nc.dram_tensor("WVKP", [P, NF * P], BF16, kind="ExternalInput")
    WOP = nc.dram_tensor("WOP", [P, NF * DM], BF16, kind="ExternalInput")
    ESEL = nc.dram_tensor("ESEL", [66, NH * 64], BF16, kind="ExternalInput")
    OUT = nc.dram_tensor("OUT", [CH, DM], F32, kind="ExternalOutput")

    with tile.TileContext(nc) as tc, ExitStack() as ctx:
        perm = ctx.enter_context(tc.tile_pool(name="perm", bufs=1))

        # warmup tile memset first so the dummy matmuls start immediately
        wtile = perm.tile([P, W], BF16, tag="wtile")
        nc.vector.memset(wtile[:], 1.0)
        identb = perm.tile([64, 64], F32, tag="identb")
        make_identity(nc, identb[:])

        # --- persistent SBUF tiles
        esel = perm.tile([66, NH * 64], BF16, tag="esel")
        wvkall = perm.tile([P, NF * P], BF16, tag="wvkall")
        wqall = perm.tile([P, NF * DM], BF16, tag="wqall")
        woall = perm.tile([P, NF * DM], BF16, tag="woall")
        qtall = perm.tile([P, NF * CH], BF16, tag="qtall")
        k3T2 = perm.tile([P, YW], BF16, tag="k3T2")
        vTs = perm.tile([64, YW], F32, tag="vTs")
        v65 = [perm.tile([P, 65], BF16, tag=f"v65_{t}", name=f"v65_{t}") for t in range(NY)]
        qpT = [perm.tile([P, CH], BF16, tag=f"qpT{m}", name=f"qpT{m}") for m in range(NF)]
        ctxn = [perm.tile([P, CH], BF16, tag=f"ctxn{i}", name=f"ctxn{i}") for i in range(NPAIR)]
        cxs = [perm.tile([64, W], BF16, tag=f"cxs{h}", name=f"cxs{h}") for h in range(NH)]
        zr16 = perm.tile([66, W], F32, tag="zr16")
        zi16 = perm.tile([66, W], F32, tag="zi16")
        zi16b = perm.tile([66, W], BF16, tag="zi16b")

        # HAM warmup: dense dummy matmuls during the DMA fill open the PE
        # clock gate (needs ~3.4us of sustained activity)
        with tc.tile_pool(name="wmps", bufs=1, space="PSUM") as wmp:
            wps = wmp.tile([P, W], F32, tag="wm")
            for _ in range(12):
                nc.tensor.matmul(wps[:], wtile[:, 0:P], wtile[:],
                                 start=True, stop=True)

        # --- input DMAs: one per tensor (kvt in 3 w-chunks)
        #   sync   : kvt chunks      scalar : wvk, qt, wq  (drains early)
        #   gpsimd : wo, esel
        nc.scalar.dma_start(wvkall[:], WVKP.ap()[:, :])
        nc.scalar.dma_start(qtall[:], QTP.ap()[:, :])
        nc.scalar.dma_start(wqall[:], WQP.ap()[:, :])
        nc.gpsimd.dma_start(esel[:], ESEL.ap()[:, :])
        nc.gpsimd.dma_start(woall[:], WOP.ap()[:, :])

        with tc.tile_pool(name="qpps", bufs=1, space="PSUM") as qpp, \
             tc.tile_pool(name="zn", bufs=6) as znp:

            def qproj(m):
                ps = qpp.tile([P, CH], F32, tag="qp")
                for f in range(NF):
                    nc.tensor.matmul(ps[:], wqall[:, DM * f + P * m:DM * f + P * (m + 1)],
                                     qtall[:, CH * f:CH * (f + 1)],
                                     start=(f == 0), stop=(f == NF - 1))
                with nc.allow_low_precision(reason="bf16 attention pipeline"):
                    nc.vector.tensor_copy(qpT[m][:], ps[:])

            with tc.tile_pool(name="kvt", bufs=1) as kvtp, \
                 tc.tile_pool(name="ph0ps", bufs=3, space="PSUM") as ph0, \
                 tc.tile_pool(name="tpps", bufs=2, space="PSUM") as tpp:
                kvtall = kvtp.tile([P, 3 * NF * W], BF16, tag="kvtall")
                for n in range(3):
                    ns_ = slice(NF * W * n, NF * W * (n + 1))
                    nc.sync.dma_start(kvtall[:, ns_], KVTP.ap()[:, ns_])

                def kvproj(n):
                    ps = ph0.tile([P, W], F32, tag="kvp")
                    base = NF * W * n
                    for f in range(NF):
                        nc.tensor.matmul(ps[:], wvkall[:, P * f:P * (f + 1)],
                                         kvtall[:, base + W * f:base + W * (f + 1)],
                                         start=(f == 0), stop=(f == NF - 1))
                    ns = slice(W * n, W * (n + 1))
                    with nc.allow_low_precision(reason="bf16 attention pipeline"):
                        nc.vector.tensor_copy(vTs[:, ns], ps[0:64, :])
                        nc.vector.tensor_copy(k3T2[64:128, ns], ps[64:128, :])

                kvproj(0)
                kvproj(1)
                kvproj(2)
                # duplicate kT into the low partition half (partition remap)
                nc.sync.dma_start(k3T2[0:64, :], k3T2[64:128, :])
                # v65 tiles: PE transpose of vT + ones column
                for t in range(NY):
                    tp = tpp.tile([P, 64], F32, tag="tp")
                    nc.tensor.transpose(tp[:], vTs[:, P * t:P * (t + 1)],
                                        identb[:])
                    with nc.allow_low_precision(reason="bf16 attention pipeline"):
                        nc.vector.tensor_copy(v65[t][:, 0:64], tp[:])
                    nc.vector.memset(v65[t][:, 64:65], 1.0)
                qproj(0)
                qproj(1)

            def z_recip(heads):
                lo, hi = _zrow(heads[0]), _zrow(heads[-1]) + 1
                with nc.allow_low_precision(reason="softmax denom"):
                    nc.vector.reciprocal(zi16[lo:hi, :], zr16[lo:hi, :])
                    nc.vector.tensor_copy(zi16b[lo:hi, :], zi16[lo:hi, :])

            def z_apply(heads):
                lo, hi = _zrow(heads[0]), _zrow(heads[-1]) + 1
                for hh in heads:
                    i, h = hh // 2, hh % 2
                    zb = qpp.tile([P, W], F32, tag="qp")
                    nc.tensor.matmul(zb[0:64, :],
                                     esel[lo:hi, 64 * hh:64 * (hh + 1)],
                                     zi16b[lo:hi, :], start=True, stop=True)
                    if h == 0:
                        with nc.allow_low_precision(reason="bf16 ctx"):
                            nc.vector.tensor_mul(ctxn[i][0:64, :], cxs[hh][:],
                                                 zb[0:64, :])
                    else:
                        cbt = znp.tile([64, W], BF16, tag="cbt")
                        with nc.allow_low_precision(reason="bf16 ctx"):
                            nc.vector.tensor_mul(cbt[:], cxs[hh][:], zb[0:64, :])
                        nc.sync.dma_start(ctxn[i][64:128, :], cbt[:])

            # --- attention per head pair; per y-tile one [128,1024] psum
            # holds both heads' scores (disjoint PE row groups -> the two
            # matmuls run concurrently), one exp ACTIVATE consumes it
            attn = ExitStack()
            scp = attn.enter_context(tc.tile_pool(name="scps", bufs=2, space="PSUM"))
            cxp = attn.enter_context(tc.tile_pool(name="cxps", bufs=3, space="PSUM"))
            ptp = attn.enter_context(tc.tile_pool(name="pt", bufs=4))
            for i in range(NPAIR):
                cxA = cxp.tile([P, W], F32, tag="cx")
                cxB = cxp.tile([P, W], F32, tag="cx")
                for y in range(NY):
                    ys = slice(P * y, P * (y + 1))
                    sc = scp.tile([P, 2 * W], F32, tag="sc")
                    nc.tensor.matmul(sc[:, 0:W], k3T2[0:64, ys],
                                     qpT[i][0:64, :], start=True, stop=True,
                                     tile_position=(0, 0))
                    nc.tensor.matmul(sc[:, W:2 * W], k3T2[64:128, ys],
                                     qpT[i][64:128, :], start=True, stop=True,
                                     tile_position=(64, 0))
                    pab = ptp.tile([P, 2 * W], BF16, tag="pt")
                    with nc.allow_low_precision(reason="bf16 probs"):
                        nc.scalar.activation(pab[:], sc[:], EXP)
                    st = (y == 0)
                    sp = (y == NY - 1)
                    nc.tensor.matmul(cxA[0:65, :], v65[y][:], pab[:, 0:W],
                                     start=st, stop=sp)
                    nc.tensor.matmul(cxB[0:65, :], v65[y][:], pab[:, W:2 * W],
                                     start=st, stop=sp)
                # stage Z row + unnormalized ctx out of PSUM (frees cx banks)
                for h, cx in ((0, cxA), (1, cxB)):
                    hh = 2 * i + h
                    zt = znp.tile([65, W], F32, tag="zt")
                    nc.vector.tensor_copy(zt[64:65, :], cx[64:65, :])
                    nc.sync.dma_start(zr16[_zrow(hh):_zrow(hh) + 1, :],
                                      zt[64:65, :])
                    with nc.allow_low_precision(reason="bf16 ctx"):
                        nc.vector.tensor_copy(cxs[hh][:], cx[0:64, :])
                if i + 2 < NF:
                    qproj(i + 2)
                if i == 5:
                    z_recip(list(range(12)))     # overlaps pair 6
                if i == 6:
                    z_apply(list(range(12)))     # overlaps pair 7
                    z_recip([12, 13])
                    z_apply([12, 13])
            attn.close()
            # heads 14,15: reciprocal overlaps the first out-proj matmuls
            z_recip([14, 15])

            # --- output projection in two 4-bank halves, i-outer
            with tc.tile_pool(name="opps", bufs=4, space="PSUM") as opp, \
                 tc.tile_pool(name="osb", bufs=4) as osb:
                allblk = [(x, o) for x in range(4) for o in range(2)]

                def oproj(pso, blocks, irange):
                    for i in irange:
                        for ps, (x, o) in zip(pso, blocks):
                            xs = slice(P * x, P * (x + 1))
                            os_ = slice(DM * i + W * o, DM * i + W * (o + 1))
                            nc.tensor.matmul(ps[:], ctxn[i][:, xs],
                                             woall[:, os_],
                                             start=(i == 0),
                                             stop=(i == NPAIR - 1))

                def drain(pso, blocks):
                    for b, (ps, (x, o)) in enumerate(zip(pso, blocks)):
                        ot = osb.tile([P, W], F32, tag="os", name=f"ot{x}_{o}")
                        if b % 2 == 0:
                            nc.scalar.copy(ot[:], ps[:])
                        else:
                            nc.vector.tensor_copy(ot[:], ps[:])
                        q = (nc.sync, nc.scalar)[b % 2]
                        q.dma_start(OUT.ap()[P * x:P * (x + 1),
                                             W * o:W * (o + 1)], ot[:])

                blocks0 = allblk[0:4]
                pso0 = [opp.tile([P, W], F32, tag="op", name=f"op0_{b}")
                        for b in range(4)]
                oproj(pso0, blocks0, range(7))
                z_apply([14, 15])
                oproj(pso0, blocks0, range(7, NPAIR))
                drain(pso0, blocks0)
                blocks1 = allblk[4:8]
                pso1 = [opp.tile([P, W], F32, tag="op", name=f"op1_{b}")
                        for b in range(4)]
                oproj(pso1, blocks1, range(NPAIR))
                drain(pso1, blocks1)

    nc.compile()
    return nc


def _get_nc():
    if "nc" not in _CACHE:
        _CACHE["nc"] = _build()
    return _CACHE["nc"]


def _esel():
    import ml_dtypes
    e = np.zeros((66, NH * 64), ml_dtypes.bfloat16)
    for h in range(NH):
        e[_zrow(h), 64 * h:64 * (h + 1)] = 1.0
    return e


def _pack(a):
    """[NF*128, C] -> [128, NF*C] with X[p, C*f + c] = a[128f + p, c]."""
    nf = a.shape[0] // P
    return np.ascontiguousarray(
        a.reshape(nf, P, a.shape[1]).transpose(1, 0, 2).reshape(P, -1))


def kernel(q, kv, Wq, Wkv, Wo, w=None, _trace=False):
    from concourse import bass_utils
    import ml_dtypes

    BF = ml_dtypes.bfloat16

    q = np.asarray(q, np.float32).reshape(L, DM)
    kv = np.asarray(kv, np.float32).reshape(L, DM)
    Wq = np.asarray(Wq, np.float32)
    Wkv = np.asarray(Wkv, np.float32)
    Wo = np.asarray(Wo, np.float32)

    qT = np.ascontiguousarray(q.T)                      # [DM, L]
    kvT = np.ascontiguousarray(kv.T)                    # [DM, L]
    WQs = Wq / np.sqrt(DH)                              # fold 1/sqrt(d_head)
    WVK = np.concatenate([Wkv[:, DH:], Wkv[:, :DH]], axis=1)  # [Wv | Wk]

    wqp = _pack(WQs).astype(BF)
    wvkp = _pack(WVK).astype(BF)
    wop = _pack(Wo).astype(BF)
    eselv = _esel()

    in_maps = []
    for c in range(NCORES):
        kvt_c = np.zeros((DM, YW), np.float32)
        lo = (c - 1) * CH
        hi = (c + 2) * CH
        src_lo, src_hi = max(lo, 0), min(hi, L)
        dst_lo = src_lo - lo
        kvt_c[:, dst_lo:dst_lo + (src_hi - src_lo)] = kvT[:, src_lo:src_hi]
        # kvtp: [128, 3*NF*W] with chunks ordered n-major then f
        kvtp = _pack(kvt_c).reshape(P, NF, 3, W).transpose(0, 2, 1, 3) \
                           .reshape(P, 3 * NF * W)
        in_maps.append({
            "QTP": _pack(qT[:, c * CH:(c + 1) * CH]).astype(BF),
            "KVTP": np.ascontiguousarray(kvtp).astype(BF),
            "WQP": wqp,
            "WVKP": wvkp,
            "WOP": wop,
            "ESEL": eselv,
        })

    nc = _get_nc()
    res = bass_utils.run_bass_kernel_spmd(
        nc, in_maps, core_ids=list(range(NCORES)), trace=_trace)
    if _trace:
        _CACHE["last_result"] = res

    out = np.concatenate([r["OUT"] for r in res.results], axis=0)
    return out.reshape(B, L, DM).astype(np.float32)
